# revision 13
# baseline (speedup 1.0000x reference)
"""Trainium2 Bass kernel for nn_ConvNet (TAGConv GNN, B=256 graphs x M=256 nodes).

Data-parallel over graphs: 32 graphs per core on 8 cores. Each graph is
fully local (kNN edges never cross graphs). Message passing is a dense
256x256 adjacency matmul per graph: with k=15 incoming edges per node the
GCN norm is exactly 1/15 per edge, folded into the hop weights host-side.
"""

import numpy as np

import concourse.bacc as bacc
import concourse.mybir as mybir
import concourse.tile as tile
from concourse.bass_utils import run_bass_kernel_spmd

N_CORES = 8
M = 256            # nodes per graph
K_NN = 15
F_IN = 7
C = 128
C2 = 6 * C         # 768
NL = 3
B_TOTAL = 256
G = B_TOTAL // N_CORES   # graphs per core
BN_EPS = 1e-5
LEAKY = 0.01
SENT = -1e30
f32 = mybir.dt.float32

_cache = {}


DEBUG = False


def _build(graphs_per_core: int):
    Gc = graphs_per_core
    nc = bacc.Bacc("TRN2", target_bir_lowering=False, debug=False,
                   num_devices=N_CORES)

    x_in = nc.dram_tensor("x", [Gc * M, F_IN], f32, kind="ExternalInput")
    wc1_in = nc.dram_tensor("wc1", [3 * F_IN, C], f32, kind="ExternalInput")
    bc1_in = nc.dram_tensor("bc1", [C, 1], f32, kind="ExternalInput")
    wc2_in = nc.dram_tensor("wc2", [3 * C, C], f32, kind="ExternalInput")
    bc2_in = nc.dram_tensor("bc2", [C, 1], f32, kind="ExternalInput")
    wc3_in = nc.dram_tensor("wc3", [3 * C, C], f32, kind="ExternalInput")
    bc3_in = nc.dram_tensor("bc3", [C, 1], f32, kind="ExternalInput")
    bng_in = nc.dram_tensor("bn_g", [C2, 1], f32, kind="ExternalInput")
    bnb_in = nc.dram_tensor("bn_b", [C2, 1], f32, kind="ExternalInput")
    w_ins = [nc.dram_tensor(f"w{i}", [C2, C2], f32, kind="ExternalInput")
             for i in range(1, 6)]
    b_ins = [nc.dram_tensor(f"b{i}", [C2, 1], f32, kind="ExternalInput")
             for i in range(1, 6)]
    wo_in = nc.dram_tensor("wo", [C2, NL], f32, kind="ExternalInput")
    bo_in = nc.dram_tensor("bo", [NL, 1], f32, kind="ExternalInput")
    y_out = nc.dram_tensor("y", [NL, Gc], f32, kind="ExternalOutput")
    dbg = {}
    if DEBUG:
        for nm, shp in [("am0", [128, M]), ("am1", [128, M]),
                        ("h1", [C, M]), ("h2", [C, M]), ("h3", [C, M]),
                        ("gt", [C2, Gc]), ("gn", [C2, Gc])]:
            dbg[nm] = nc.dram_tensor("dbg_" + nm, shp, f32, kind="ExternalOutput")

    from contextlib import ExitStack
    with tile.TileContext(nc) as tc, ExitStack() as stack:
        wpool = stack.enter_context(tc.tile_pool(name="weights", bufs=1))
        cpool = stack.enter_context(tc.tile_pool(name="consts", bufs=1))
        gpool = stack.enter_context(tc.tile_pool(name="gfeat", bufs=1))
        pool = stack.enter_context(tc.tile_pool(name="work", bufs=3))
        pool2 = stack.enter_context(tc.tile_pool(name="work2", bufs=4))
        dpool = stack.enter_context(tc.tile_pool(name="dram", bufs=1, space="DRAM"))
        ps_big = stack.enter_context(tc.tile_pool(name="ps_big", bufs=4, space="PSUM"))
        ps_med = stack.enter_context(tc.tile_pool(name="ps_med", bufs=4, space="PSUM"))
        ps_sm = ps_med

        # ---- constants ----
        ident = cpool.tile([128, 128], f32, tag="ident")
        nc.vector.memset(ident[:, :], 1.0)
        nc.gpsimd.affine_select(ident[:, :], ident[:, :], pattern=[[1, 128]],
                                compare_op=mybir.AluOpType.is_equal,
                                fill=0.0, base=0, channel_multiplier=-1)
        ones4 = cpool.tile([4, 1], f32, tag="ones4")
        nc.vector.memset(ones4[:, :], 1.0)
        onesrow = cpool.tile([1, 256], f32, tag="onesrow")
        nc.vector.memset(onesrow[:, :], 1.0)
        negrow = cpool.tile([1, 256], f32, tag="negrow")
        nc.vector.memset(negrow[:, :], -1.0)
        epsc = cpool.tile([C, 1], f32, tag="epsc")
        nc.vector.memset(epsc[:, :], BN_EPS)

        # ---- conv weights / biases ----
        wc1 = []
        for k in range(3):
            t_ = wpool.tile([F_IN, C], f32, tag=f"wc1_{k}", name=f"wc1_{k}")
            nc.sync.dma_start(t_[:, :], wc1_in[k * F_IN:(k + 1) * F_IN, :])
            wc1.append(t_)
        wc2 = []
        for k in range(3):
            t_ = wpool.tile([C, C], f32, tag=f"wc2_{k}")
            nc.sync.dma_start(t_[:, :], wc2_in[k * C:(k + 1) * C, :])
            wc2.append(t_)
        wc3 = []
        for k in range(3):
            t_ = wpool.tile([C, C], f32, tag=f"wc3_{k}")
            nc.sync.dma_start(t_[:, :], wc3_in[k * C:(k + 1) * C, :])
            wc3.append(t_)
        bc1 = wpool.tile([C, 1], f32, tag="bc1")
        nc.sync.dma_start(bc1[:, :], bc1_in[:, :])
        bc2 = wpool.tile([C, 1], f32, tag="bc2")
        nc.sync.dma_start(bc2[:, :], bc2_in[:, :])
        bc3 = wpool.tile([C, 1], f32, tag="bc3")
        nc.sync.dma_start(bc3[:, :], bc3_in[:, :])

        # graph-level feature accumulators [128, Gc] x 6 (feature-major)
        gt = [gpool.tile([C, Gc], f32, tag=f"gt{i}", name=f"gt{i}") for i in range(6)]

        # ---- MLP weights (issued early; consumed at the end) ----
        w_sb = []
        for li in range(5):
            blocks = []
            for ci in range(6):
                wt = wpool.tile([C, C2], f32, tag=f"w{li}_{ci}")
                nc.sync.dma_start(wt[:, :], w_ins[li][ci * C:(ci + 1) * C, :])
                blocks.append(wt)
            w_sb.append(blocks)
        # biases: load as six [128,1] tiles per layer
        b_sb = []
        for li in range(5):
            tiles = []
            for ci in range(6):
                bt = wpool.tile([C, 1], f32, tag=f"bl{li}_{ci}")
                nc.sync.dma_start(bt[:, :], b_ins[li][ci * C:(ci + 1) * C, :])
                tiles.append(bt)
            b_sb.append(tiles)
        wo_sb = []
        for ci in range(6):
            wt = wpool.tile([C, NL], f32, tag=f"wo_{ci}")
            nc.sync.dma_start(wt[:, :], wo_in[ci * C:(ci + 1) * C, :])
            wo_sb.append(wt)
        bo_sb = wpool.tile([NL, 1], f32, tag="bo")
        nc.sync.dma_start(bo_sb[:, :], bo_in[:, :])
        bng_sb = []
        bnb_sb = []
        for ci in range(6):
            g_t = wpool.tile([C, 1], f32, tag=f"bng_{ci}")
            nc.sync.dma_start(g_t[:, :], bng_in[ci * C:(ci + 1) * C, :])
            bng_sb.append(g_t)
            b_t = wpool.tile([C, 1], f32, tag=f"bnb_{ci}")
            nc.sync.dma_start(b_t[:, :], bnb_in[ci * C:(ci + 1) * C, :])
            bnb_sb.append(b_t)

        # ================= per-graph conv pipeline =================
        for g in range(Gc):
            base = g * M
            # node-major x chunks [128, 7]
            xnm = []
            for t in range(2):
                xt_ = pool2.tile([128, F_IN], f32, tag="xnm")
                nc.sync.dma_start(xt_[:, :],
                                  x_in[base + t * 128: base + (t + 1) * 128, :])
                xnm.append(xt_)
            # feature-major xT [7, 256] via PE transpose
            xT = pool.tile([F_IN, M], f32, tag="xT")
            for t in range(2):
                pst = ps_sm.tile([F_IN, 128], f32, tag="psmed")
                nc.tensor.transpose(pst[:, :], xnm[t][:, :], ident[:, :])
                nc.scalar.activation(xT[:, t * 128:(t + 1) * 128], pst[:, :],
                                     mybir.ActivationFunctionType.Copy)

            # ---- kNN: s = -d2 = 2*pos@pos.T - sq_i - sq_j ----
            p2 = pool.tile([4, M], f32, tag="p2")
            nc.vector.tensor_tensor(p2[:, :], xT[0:4, :], xT[0:4, :],
                                    mybir.AluOpType.mult)
            pos2x = pool.tile([4, M], f32, tag="pos2x")
            nc.scalar.activation(pos2x[:, :], xT[0:4, :],
                                 mybir.ActivationFunctionType.Copy, scale=2.0)
            sq_ps = ps_sm.tile([1, M], f32, tag="psmed")
            nc.tensor.matmul(sq_ps[:, :], ones4[:, :], p2[:, :],
                             start=True, stop=True)
            sqrow = pool.tile([1, M], f32, tag="sqrow")
            nc.scalar.activation(sqrow[:, :], sq_ps[:, :],
                                 mybir.ActivationFunctionType.Copy)
            nsqrow = pool.tile([1, M], f32, tag="nsqrow")
            nc.scalar.activation(nsqrow[:, :], sq_ps[:, :],
                                 mybir.ActivationFunctionType.Copy, scale=-1.0)

            Amask = []
            for t in range(2):
                s_ps = ps_big.tile([128, M], f32, tag="psbig")
                sl = slice(t * 128, (t + 1) * 128)
                nc.tensor.matmul(s_ps[:, :], xT[0:4, sl], pos2x[:, :],
                                 start=True, stop=False)
                nc.tensor.matmul(s_ps[:, :], onesrow[:, 0:128], nsqrow[:, :],
                                 start=False, stop=False)
                nc.tensor.matmul(s_ps[:, :], sqrow[:, sl], negrow[:, :],
                                 start=False, stop=True)
                s_sb = pool2.tile([128, M], f32, tag="s_sb")
                nc.scalar.activation(s_sb[:, :], s_ps[:, :],
                                     mybir.ActivationFunctionType.Copy)
                # self-loop exclusion: s[i, i] = -1e10
                nc.gpsimd.affine_select(s_sb[:, :], s_sb[:, :],
                                        pattern=[[1, M]],
                                        compare_op=mybir.AluOpType.not_equal,
                                        fill=-1e10, base=-(t * 128),
                                        channel_multiplier=-1)
                # top-15 selection -> sentinel marks
                m8a = pool2.tile([128, 8], f32, tag="m8a")
                nc.vector.max(m8a[:, :], s_sb[:, :])
                s1 = pool2.tile([128, M], f32, tag="s1")
                nc.vector.match_replace(s1[:, :], m8a[:, :], s_sb[:, :], SENT)
                m8b = pool2.tile([128, 8], f32, tag="m8b")
                nc.vector.max(m8b[:, :], s1[:, :])
                nc.vector.memset(m8b[:, 7:8], 1e30)
                s2 = pool2.tile([128, M], f32, tag="s2")
                nc.vector.match_replace(s2[:, :], m8b[:, :], s1[:, :], SENT)
                am = pool2.tile([128, M], f32, tag="Amask")
                nc.gpsimd.tensor_scalar(am[:, :], s2[:, :], SENT, None,
                                        mybir.AluOpType.is_equal)
                if DEBUG and g == 0:
                    nc.sync.dma_start(dbg["am" + str(t)][:, :], am[:, :])
                Amask.append(am)

            # ---- AT = A^T chunks [src 128, dst 256] x2 via PE transpose ----
            AT = []
            for st in range(2):
                at = pool2.tile([128, M], f32, tag="AT")
                for dt_ in range(2):
                    pst = ps_med.tile([128, 128], f32, tag="psmed")
                    nc.tensor.transpose(pst[:, :],
                                        Amask[dt_][:, st * 128:(st + 1) * 128],
                                        ident[:, :])
                    dsl = slice(dt_ * 128, (dt_ + 1) * 128)
                    if dt_ == 0:
                        nc.scalar.activation(at[:, dsl], pst[:, :],
                                             mybir.ActivationFunctionType.Copy)
                    else:
                        nc.vector.tensor_copy(at[:, dsl], pst[:, :])
                AT.append(at)

            # ---- Layer 1 (F_IN=7 -> C) ----
            m1f_ps = ps_big.tile([F_IN, M], f32, tag="psbig")
            nc.tensor.matmul(m1f_ps[:, :], xnm[0][:, :], AT[0][:, :],
                             start=True, stop=False)
            nc.tensor.matmul(m1f_ps[:, :], xnm[1][:, :], AT[1][:, :],
                             start=False, stop=True)
            m1f = pool.tile([F_IN, M], f32, tag="m1f")
            nc.vector.tensor_copy(m1f[:, :], m1f_ps[:, :])

            m1n = []
            for dt_ in range(2):
                mn_ps = ps_sm.tile([128, F_IN], f32, tag="psmed")
                dsl = slice(dt_ * 128, (dt_ + 1) * 128)
                nc.tensor.matmul(mn_ps[:, :], AT[0][:, dsl], xnm[0][:, :],
                                 start=True, stop=False)
                nc.tensor.matmul(mn_ps[:, :], AT[1][:, dsl], xnm[1][:, :],
                                 start=False, stop=True)
                mn = pool.tile([128, F_IN], f32, tag="m1n")
                nc.vector.tensor_copy(mn[:, :], mn_ps[:, :])
                m1n.append(mn)

            m2f_ps = ps_big.tile([F_IN, M], f32, tag="psbig")
            nc.tensor.matmul(m2f_ps[:, :], m1n[0][:, :], AT[0][:, :],
                             start=True, stop=False)
            nc.tensor.matmul(m2f_ps[:, :], m1n[1][:, :], AT[1][:, :],
                             start=False, stop=True)
            m2f = pool.tile([F_IN, M], f32, tag="m2f")
            nc.scalar.activation(m2f[:, :], m2f_ps[:, :],
                                 mybir.ActivationFunctionType.Copy)

            h_ps = ps_big.tile([C, M], f32, tag="psbig")
            nc.tensor.matmul(h_ps[:, :], wc1[0][:, :], xT[:, :],
                             start=True, stop=False)
            nc.tensor.matmul(h_ps[:, :], wc1[1][:, :], m1f[:, :],
                             start=False, stop=False)
            nc.tensor.matmul(h_ps[:, :], wc1[2][:, :], m2f[:, :],
                             start=False, stop=True)
            h1 = pool2.tile([C, M], f32, tag="h1")
            nc.scalar.activation(h1[:, :], h_ps[:, :],
                                 mybir.ActivationFunctionType.Lrelu,
                                 bias=bc1[:, 0:1], alpha=LEAKY,
                                 accum_out=gt[0][:, g:g + 1])
            nc.vector.tensor_reduce(gt[1][:, g:g + 1], h1[:, :],
                                    mybir.AxisListType.X, mybir.AluOpType.max)
            if DEBUG and g == 0:
                nc.sync.dma_start(dbg["h1"][:, :], h1[:, :])

            # ---- Layers 2 and 3 (C -> C) ----
            h_prev = h1
            for li, (wc, bcn, gm, gx) in enumerate(
                    [(wc2, bc2, 2, 3), (wc3, bc3, 4, 5)]):
                # node-major chunks of h_prev
                hn = []
                for t in range(2):
                    pst = ps_med.tile([128, 128], f32, tag="psmed")
                    nc.tensor.transpose(pst[:, :],
                                        h_prev[:, t * 128:(t + 1) * 128],
                                        ident[:, :])
                    hnt = pool.tile([128, C], f32, tag="hn")
                    if t == 0:
                        nc.scalar.activation(hnt[:, :], pst[:, :],
                                             mybir.ActivationFunctionType.Copy)
                    else:
                        nc.vector.tensor_copy(hnt[:, :], pst[:, :])
                    hn.append(hnt)

                m1f_ps = ps_big.tile([C, M], f32, tag="psbig")
                nc.tensor.matmul(m1f_ps[:, :], hn[0][:, :], AT[0][:, :],
                                 start=True, stop=False)
                nc.tensor.matmul(m1f_ps[:, :], hn[1][:, :], AT[1][:, :],
                                 start=False, stop=True)
                m1fc = pool.tile([C, M], f32, tag="m1fc")
                nc.vector.tensor_copy(m1fc[:, :], m1f_ps[:, :])

                m1nc = []
                for t in range(2):
                    pst = ps_med.tile([128, 128], f32, tag="psmed")
                    nc.tensor.transpose(pst[:, :],
                                        m1fc[:, t * 128:(t + 1) * 128],
                                        ident[:, :])
                    mnt = pool.tile([128, C], f32, tag="m1nc")
                    if t == 0:
                        nc.scalar.activation(mnt[:, :], pst[:, :],
                                             mybir.ActivationFunctionType.Copy)
                    else:
                        nc.vector.tensor_copy(mnt[:, :], pst[:, :])
                    m1nc.append(mnt)

                m2f_ps = ps_big.tile([C, M], f32, tag="psbig")
                nc.tensor.matmul(m2f_ps[:, :], m1nc[0][:, :], AT[0][:, :],
                                 start=True, stop=False)
                nc.tensor.matmul(m2f_ps[:, :], m1nc[1][:, :], AT[1][:, :],
                                 start=False, stop=True)
                m2fc = pool.tile([C, M], f32, tag="m2fc")
                nc.scalar.activation(m2fc[:, :], m2f_ps[:, :],
                                     mybir.ActivationFunctionType.Copy)

                h_ps = ps_big.tile([C, M], f32, tag="psbig")
                nc.tensor.matmul(h_ps[:, :], wc[0][:, :], h_prev[:, :],
                                 start=True, stop=False)
                nc.tensor.matmul(h_ps[:, :], wc[1][:, :], m1fc[:, :],
                                 start=False, stop=False)
                nc.tensor.matmul(h_ps[:, :], wc[2][:, :], m2fc[:, :],
                                 start=False, stop=True)
                hnew = pool2.tile([C, M], f32, tag=f"h{li + 2}")
                nc.scalar.activation(hnew[:, :], h_ps[:, :],
                                     mybir.ActivationFunctionType.Lrelu,
                                     bias=bcn[:, 0:1], alpha=LEAKY,
                                     accum_out=gt[gm][:, g:g + 1])
                nc.vector.tensor_reduce(gt[gx][:, g:g + 1], hnew[:, :],
                                        mybir.AxisListType.X,
                                        mybir.AluOpType.max)
                if DEBUG and g == 0:
                    nc.sync.dma_start(dbg["h" + str(li + 2)][:, :], hnew[:, :])
                h_prev = hnew

        # ================= BatchNorm over full batch =================
        # mean-pool columns hold raw node sums; rescale to true means so the
        # BN eps acts at the reference scale
        for i in (0, 2, 4):
            nc.vector.tensor_scalar_mul(gt[i][:, :], gt[i][:, :], 1.0 / M)
        stats_in = dpool.tile([C2, 2], f32, tag="stats_in")
        stats_out = dpool.tile([C2, 2], f32, tag="stats_out")
        for i in range(6):
            st = pool.tile([C, 2], f32, tag="stloc")
            nc.vector.tensor_reduce(st[:, 0:1], gt[i][:, :],
                                    mybir.AxisListType.X, mybir.AluOpType.add)
            junk = pool.tile([C, Gc], f32, tag="junk")
            nc.scalar.activation(junk[:, :], gt[i][:, :],
                                 mybir.ActivationFunctionType.Square,
                                 accum_out=st[:, 1:2])
            nc.sync.dma_start(stats_in[i * C:(i + 1) * C, :], st[:, :])
        nc.gpsimd.collective_compute(
            "AllReduce", mybir.AluOpType.add,
            replica_groups=[list(range(N_CORES))],
            ins=[stats_in.opt()], outs=[stats_out.opt()])

        inv_b = 1.0 / (N_CORES * Gc)
        gn = []
        for i in range(6):
            st = pool.tile([C, 2], f32, tag="stred")
            nc.sync.dma_start(st[:, :], stats_out[i * C:(i + 1) * C, :])
            mu = pool.tile([C, 1], f32, tag="mu")
            nc.vector.tensor_scalar_mul(mu[:, :], st[:, 0:1], inv_b)
            ex2 = pool.tile([C, 1], f32, tag="ex2")
            nc.vector.tensor_scalar_mul(ex2[:, :], st[:, 1:2], inv_b)
            musq = pool.tile([C, 1], f32, tag="musq")
            nc.vector.tensor_tensor(musq[:, :], mu[:, :], mu[:, :],
                                    mybir.AluOpType.mult)
            var = pool.tile([C, 1], f32, tag="var")
            nc.vector.tensor_tensor(var[:, :], ex2[:, :], musq[:, :],
                                    mybir.AluOpType.subtract)
            sd = pool.tile([C, 1], f32, tag="sd")
            nc.scalar.activation(sd[:, :], var[:, :],
                                 mybir.ActivationFunctionType.Sqrt,
                                 bias=epsc[:, 0:1])
            inv = pool.tile([C, 1], f32, tag="inv")
            nc.vector.reciprocal(inv[:, :], sd[:, :])
            scl = pool.tile([C, 1], f32, tag="scl")
            nc.vector.tensor_tensor(scl[:, :], inv[:, :], bng_sb[i][:, :],
                                    mybir.AluOpType.mult)
            msc = pool.tile([C, 1], f32, tag="msc")
            nc.vector.tensor_tensor(msc[:, :], mu[:, :], scl[:, :],
                                    mybir.AluOpType.mult)
            shf = pool.tile([C, 1], f32, tag="shf")
            nc.vector.tensor_tensor(shf[:, :], bnb_sb[i][:, :], msc[:, :],
                                    mybir.AluOpType.subtract)
            gni = pool.tile([C, Gc], f32, tag=f"gn{i}")
            nc.scalar.activation(gni[:, :], gt[i][:, :],
                                 mybir.ActivationFunctionType.Identity,
                                 bias=shf[:, 0:1], scale=scl[:, 0:1])
            gn.append(gni)

        if DEBUG:
            for i in range(6):
                nc.sync.dma_start(dbg["gt"][i * C:(i + 1) * C, :], gt[i][:, :])
                nc.sync.dma_start(dbg["gn"][i * C:(i + 1) * C, :], gn[i][:, :])

        # ================= MLP head =================
        cur = gn
        for li in range(5):
            nxt = []
            for co in range(6):
                ps = ps_med.tile([C, Gc], f32, tag="psmed")
                for ci in range(6):
                    nc.tensor.matmul(ps[:, :],
                                     w_sb[li][ci][:, co * C:(co + 1) * C],
                                     cur[ci][:, :],
                                     start=(ci == 0), stop=(ci == 5))
                nt = pool.tile([C, Gc], f32, tag=f"mlp{li}_{co}")
                nc.scalar.activation(nt[:, :], ps[:, :],
                                     mybir.ActivationFunctionType.Lrelu,
                                     bias=b_sb[li][co][:, 0:1], alpha=LEAKY)
                nxt.append(nt)
            cur = nxt

        ps_o = ps_sm.tile([NL, Gc], f32, tag="psmed")
        for ci in range(6):
            nc.tensor.matmul(ps_o[:, :], wo_sb[ci][:, :], cur[ci][:, :],
                             start=(ci == 0), stop=(ci == 5))
        y_sb = pool.tile([NL, Gc], f32, tag="y_sb")
        nc.scalar.activation(y_sb[:, :], ps_o[:, :],
                             mybir.ActivationFunctionType.Identity,
                             bias=bo_sb[:, 0:1])
        yt = pool.tile([2, Gc], f32, tag="yt")
        nc.scalar.activation(yt[:, :], y_sb[0:2, :],
                             mybir.ActivationFunctionType.Tanh)
        nc.sync.dma_start(y_out[0:2, :], yt[:, :])
        nc.sync.dma_start(y_out[2:3, :], y_sb[2:3, :])

    nc.compile()
    return nc


def _prep_weights(inputs):
    """Host-side: fold GCN norm (1/15 per hop) into hop weights, reshape."""
    def conv_w(w):  # [3, F, C] -> [3F, C] with hop scaling
        w = np.asarray(w, dtype=np.float32).copy()
        w[1] /= K_NN
        w[2] /= K_NN * K_NN
        return w.reshape(-1, w.shape[-1])

    m = {
        "wc1": conv_w(inputs["Wc1"]),
        "bc1": np.asarray(inputs["bc1"], np.float32).reshape(C, 1),
        "wc2": conv_w(inputs["Wc2"]),
        "bc2": np.asarray(inputs["bc2"], np.float32).reshape(C, 1),
        "wc3": conv_w(inputs["Wc3"]),
        "bc3": np.asarray(inputs["bc3"], np.float32).reshape(C, 1),
        "bn_g": np.asarray(inputs["bn_g"], np.float32).reshape(C2, 1),
        "bn_b": np.asarray(inputs["bn_b"], np.float32).reshape(C2, 1),
        "wo": np.asarray(inputs["Wo"], np.float32),
        "bo": np.asarray(inputs["bo"], np.float32).reshape(NL, 1),
    }
    for i in range(1, 6):
        m[f"w{i}"] = np.asarray(inputs[f"W{i}"], np.float32)
        m[f"b{i}"] = np.asarray(inputs[f"b{i}"], np.float32).reshape(C2, 1)
    return m


def get_nc(graphs_per_core=G):
    if graphs_per_core not in _cache:
        _cache[graphs_per_core] = _build(graphs_per_core)
    return _cache[graphs_per_core]


def make_in_maps(inputs, graphs_per_core=G):
    x = np.asarray(inputs["x"], np.float32)
    w = _prep_weights(inputs)
    in_maps = []
    for c in range(N_CORES):
        base = c * graphs_per_core * M
        m = dict(w)
        m["x"] = x[base: base + graphs_per_core * M]
        in_maps.append(m)
    return in_maps


def kernel(**inputs):
    nc = get_nc(G)
    in_maps = make_in_maps(inputs, G)
    res = run_bass_kernel_spmd(nc, in_maps, list(range(N_CORES)))
    y = np.concatenate([res.results[c]["y"].T for c in range(N_CORES)], axis=0)
    return y.astype(np.float32)
